# revision 57
# baseline (speedup 1.0000x reference)
"""Trainium2 Bass kernel for a 2-layer GCN (GCNConv -> ReLU -> GCNConv).

Math (reference):
    add self-loops; deg = indegree (unit weights); dis = deg^-1/2
    norm_e = dis[row_e] * dis[col_e]
    h   = relu( segsum_col( (x @ W1)[row] * norm ) + b1 )
    out =       segsum_col( (h @ W2)[row] * norm ) + b2

Kernel reorganization (linearity + norm factorization):
    norm factors as dis[row] * dis[col]:
      - dis[row] is folded into the gathered tables host-side:
        xs[v] = x[v] * dis[v] (bf16), and on device hw[v] = dis[v]*(h[v] @ W2).
      - dis[col] is applied per destination at PSUM eviction (per-partition
        scale), so the edge selection matrix is a pure 0/1 one-hot built with
        a single-op is_equal tensor_scalar (~3.5x cheaper than the fused
        one-hot*norm build).
    agg[d]  = sum_e xs[row_e]          (one-hot matmul segment sum)
    h[d]    = relu( dis[d] * (agg[d] @ W1) + b1 )
    hw[v]   = dis[v] * (h[v] @ W2)     (bf16 table, AllGather'd)
    out[d]  = dis[d] * sum_e hw[row_e] + b2

Distribution (8 cores, SPMD shared program): destinations sharded
contiguously (12500 nodes/core); both layers are dest-sharded gathers +
on-chip one-hot-matmul segment reduction.  Both layers share ONE slot
layout / index metadata: tables are padded to 12544 rows per core
(ghw = owner*12544 + local), so layer 1 (xs table) and layer 2 (hw table)
gather with identical int16 bank indices.

Self-loops bypass the gather: per-tile contributions come from a
sequential load (xsloc, lane-major) / the SBUF-resident hw tiles, via an
identity matmul.  Edge slots are packed contiguously per (batch, bank)
with boundary slots shared between adjacent tiles (one sel column per
(tile, slot) pair), minimizing gathered descriptors — the kernel is
bound by the SWDGE gather descriptor service rate (~2-4 ns/row).
Between layers: AllGather of the compact [12544, 40] bf16 hw tiles, then
an on-chip restride (DVE) to the 256B-row gather table layout.
"""

import os
import sys

for _p in ("/opt/trn_rl_repo", "/root/.axon_site/_ro/trn_rl_repo"):
    if os.path.isdir(_p) and _p not in sys.path:
        sys.path.insert(0, _p)

import numpy as np

P = 128
BK = 32768           # int16 bank rows
CALL_SLOTS = int(os.environ.get("GCN_CALL_SLOTS", "8"))
NQ = 4               # SWDGE queues


class Plan:
    pass


def _layout(nmax, T, NB, batch_cap):
    """Contiguous slot stream: per batch, per bank, tiles' edge runs are
    packed back-to-back (no per-tile alignment); only the batch-bank run is
    padded to a slot multiple.  Boundary slots shared by two tiles get one
    sel column per (tile, slot) pair.

    nmax: [T, NB] regularized (max-over-cores) edge counts.
    Returns (S, NSEL, batches, estart): batches carry
      tile_lo, tile_hi, slot_lo, slot_hi,
      calls: [(slot_lo, nslots, bank)],
      tile_work: {t: [(selcol, slot), ...]},
    and estart[T, NB] = stream position (global lane index) of each
    (tile, bank) edge run.
    """
    tile_tot = nmax.sum(axis=1)
    batches = []
    estart = np.zeros((T, NB), dtype=np.int64)
    gslot = 0
    nsel = 0
    i = 0
    while i < T:
        # batch sizing by estimated slots
        j = i + 1
        tot = -(-tile_tot[i] // P)
        while j < T and tot + (-(-tile_tot[j] // P)) <= batch_cap:
            tot += -(-tile_tot[j] // P)
            j += 1
        b0 = {"tile_lo": i, "tile_hi": j, "slot_lo": gslot,
              "calls": [], "tile_work": {k: [] for k in range(i, j)}}
        for b in range(NB):
            run_slot_lo = gslot
            pos = 0
            for k in range(i, j):
                n = int(nmax[k, b])
                if n == 0:
                    continue
                estart[k, b] = run_slot_lo * P + pos
                s0 = pos // P
                s1 = -(-(pos + n) // P)
                for s in range(s0, s1):
                    b0["tile_work"][k].append((nsel, run_slot_lo + s))
                    nsel += 1
                pos += n
            nrun = -(-pos // P)
            gslot = run_slot_lo + nrun
            r = run_slot_lo
            while r < gslot:
                n = min(CALL_SLOTS, gslot - r)
                b0["calls"].append((r, n, b))
                r += n
        b0["slot_hi"] = gslot
        batches.append(b0)
        i = j
    return int(gslot), int(nsel), batches, estart


def make_plan(edge_index, n_nodes, n_cores, f_in, hidden, n_class,
              batch_cap=int(os.environ.get("GCN_BATCH_CAP", "96"))):
    pl = Plan()
    N, M = n_nodes, n_cores
    row = np.asarray(edge_index[0], dtype=np.int64)
    col = np.asarray(edge_index[1], dtype=np.int64)
    loops = np.arange(N, dtype=np.int64)
    deg = np.bincount(np.concatenate([col, loops]), minlength=N).astype(np.float32)
    # self-loops are handled by a direct sequential load + identity matmul,
    # not the gather stream
    row_all = row
    col_all = col
    dis = (1.0 / np.sqrt(np.maximum(deg, 1e-12))).astype(np.float32)
    dis[deg <= 0] = 0.0

    Nc = -(-N // M)
    T = -(-Nc // P)
    GR = T * P                      # padded rows per core
    NR = M * GR                     # padded table rows
    NB = -(-NR // BK)

    owner = col_all // Nc
    local = col_all - owner * Nc
    ltile = local // P
    colrel = (local - ltile * P).astype(np.float32)

    # shared (both layers) gather index: padded per-core row space
    src_owner = row_all // Nc
    ghw = src_owner * GR + (row_all - src_owner * Nc)
    bank = ghw // BK
    lidx = (ghw - bank * BK).astype(np.int16)

    cc = np.zeros((M, T, NB), dtype=np.int64)
    np.add.at(cc, (owner, ltile, bank), 1)
    nmax = cc.max(axis=0)
    S, NSEL, batches, estart = _layout(nmax, T, NB, batch_cap)
    col_of = np.full((T, S), -1, dtype=np.int64)
    for bat in batches:
        for t, work in bat["tile_work"].items():
            for (colid, s) in work:
                col_of[t, s] = colid

    # scatter per-edge metadata into slot/lane arrays
    E2 = row_all.shape[0]
    blockid = (owner * T + ltile) * NB + bank
    counts = np.bincount(blockid, minlength=M * T * NB)
    order = np.argsort(blockid, kind="stable")
    sb = blockid[order]
    starts = np.zeros(M * T * NB + 1, dtype=np.int64)
    np.cumsum(counts, out=starts[1:])
    q = np.arange(E2, dtype=np.int64) - starts[sb]
    o_tile = ltile[order]
    o_bank = bank[order]
    e = estart[o_tile, o_bank] + q
    slot = e // P
    lane = e % P

    colsel = np.full((M, P, NSEL), -1.0, dtype=np.float32)
    g16 = np.zeros((M, 16, 8 * S), dtype=np.int16)
    o_owner = owner[order]
    colsel[o_owner, lane, col_of[o_tile, slot]] = colrel[order]
    g16[o_owner, e % 16, e // 16] = lidx[order]

    # per-destination dis (by tile/lane) and its inverse row
    v = np.arange(M * GR, dtype=np.int64)
    vc = v // GR
    vl = v - vc * GR
    node = vc * Nc + np.minimum(vl, Nc - 1)
    disp = dis[node].copy()
    disp[vl >= Nc] = 1.0
    dis_c = disp.reshape(M, T, P).transpose(0, 2, 1)          # [M, P, T]
    dcinv = (1.0 / disp).reshape(M, 1, GR)                    # [M, 1, T*P]

    pl.N, pl.M, pl.Nc, pl.T = N, M, Nc, T
    pl.GR, pl.NR, pl.NB, pl.S = GR, NR, NB, S
    pl.NSEL = NSEL
    pl.F, pl.H, pl.C = f_in, hidden, n_class
    pl.dis = dis
    pl.batches = batches
    pl.colsel = colsel
    pl.gidx16 = np.tile(g16, (1, 8, 1))
    pl.dis_c = np.ascontiguousarray(dis_c)
    pl.dcinv = np.ascontiguousarray(dcinv)
    return pl


# ---------------------------------------------------------------------------
# Device program
# ---------------------------------------------------------------------------
def build_program(pl, debug=False):
    from concourse import bass, bacc, mybir
    import concourse.tile as tile
    from contextlib import ExitStack

    f32 = mybir.dt.float32
    bf16 = mybir.dt.bfloat16
    i32 = mybir.dt.int32
    i16 = mybir.dt.int16
    N, M, T = pl.N, pl.M, pl.T
    F, H, C = pl.F, pl.H, pl.C
    GR, NR, S, NSEL = pl.GR, pl.NR, pl.S, pl.NSEL

    nc = bacc.Bacc("TRN2", target_bir_lowering=False, debug=debug,
                   num_devices=M, num_swdge_queues=NQ)
    xs_p = nc.declare_dram_parameter("xs", [NR, F], bf16, isOutput=False)
    xsloc_p = nc.declare_dram_parameter("xsloc", [P, T * F], bf16, isOutput=False)
    w1_p = nc.declare_dram_parameter("W1b", [F, H], bf16, isOutput=False)
    b1_p = nc.declare_dram_parameter("b1b", [1, H], bf16, isOutput=False)
    w2_p = nc.declare_dram_parameter("W2b", [H, C], bf16, isOutput=False)
    b2_p = nc.declare_dram_parameter("b2b", [1, C], bf16, isOutput=False)
    colsel_p = nc.declare_dram_parameter("colsel", [P, NSEL], bf16, isOutput=False)
    g16_p = nc.declare_dram_parameter("g16", [P, 8 * S], i16, isOutput=False)
    disc_p = nc.declare_dram_parameter("disc", [P, T], f32, isOutput=False)
    dcinv_p = nc.declare_dram_parameter("dcinv", [1, GR], bf16, isOutput=False)
    out_p = nc.declare_dram_parameter("out", [P, T * C], f32, isOutput=True)

    hw_ag_in = nc.dram_tensor("hw_ag_in", [GR, C], bf16)
    hw_ag_out = nc.dram_tensor("hw_ag_out", [NR, C], bf16, addr_space="Shared")
    # per-bank tables so layer-2 gathers unblock per repacked bank
    bank_rows = [min(BK, NR - b * BK) for b in range(pl.NB)]
    hw_tab_b = [nc.dram_tensor(f"hw_tab{b}", [bank_rows[b], P], bf16)
                for b in range(pl.NB)]

    qrr = [0]

    def next_q():
        q = qrr[0]
        qrr[0] = (q + 1) % NQ
        return q

    with tile.TileContext(nc) as tc, ExitStack() as ctx:
        const = ctx.enter_context(tc.tile_pool(name="const", bufs=1))

        iota_i = const.tile([P, P], i32)
        nc.gpsimd.iota(iota_i[:], pattern=[[1, P]], base=0, channel_multiplier=0)
        iota_b = const.tile([P, P], bf16)
        nc.vector.tensor_copy(out=iota_b[:], in_=iota_i[:])
        SELBLK = 8
        iota_b8 = const.tile([P, SELBLK * P], bf16)
        for k in range(SELBLK):
            nc.vector.tensor_copy(out=iota_b8[:, k * P:(k + 1) * P], in_=iota_b[:])
        iotac_i = const.tile([P, 1], i32)
        nc.gpsimd.iota(iotac_i[:], pattern=[[1, 1]], base=0, channel_multiplier=1)
        iotac_f = const.tile([P, 1], f32)
        nc.vector.tensor_copy(out=iotac_f[:], in_=iotac_i[:])
        ident_b = const.tile([P, P], bf16)
        nc.vector.tensor_scalar(out=ident_b[:], in0=iota_b[:],
                                scalar1=iotac_f[:], scalar2=None,
                                op0=mybir.AluOpType.is_equal)

        w1_sb = const.tile([F, H], bf16)
        b1_sb = const.tile([1, H], bf16)
        w2_sb = const.tile([H, C], bf16)
        b2_sb = const.tile([1, C], bf16)
        disc_sb = const.tile([P, T], f32)
        dcinv_sb = const.tile([1, GR], bf16)
        nc.sync.dma_start(out=w1_sb[:], in_=w1_p[:, :])
        nc.sync.dma_start(out=b1_sb[:], in_=b1_p[:, :])
        nc.sync.dma_start(out=w2_sb[:], in_=w2_p[:, :])
        nc.sync.dma_start(out=b2_sb[:], in_=b2_p[:, :])
        nc.sync.dma_start(out=disc_sb[:], in_=disc_p[:, :])
        nc.sync.dma_start(out=dcinv_sb[:], in_=dcinv_p[:, :])

        colsel_sb = const.tile([P, NSEL], bf16)
        g16_sb = const.tile([P, 8 * S], i16)
        nc.sync.dma_start(out=colsel_sb[:], in_=colsel_p[:, :])
        nc.sync.dma_start(out=g16_sb[:], in_=g16_p[:, :])
        # all hw tiles stay SBUF-resident for layer 2's self-loop term
        hwall_sb = const.tile([P, T * C], bf16)
        # outputs accumulate lane-major in SBUF; one bulk store at the end
        outall_sb = const.tile([P, T * C], f32)

        def sel_build_run(pool, col0, k):
            """One DVE op builds sel matrices for k consecutive sel columns."""
            selB = pool.tile([P, SELBLK * P], bf16, name="selB")
            src = colsel_sb[:, col0:col0 + k]
            bc = bass.AP(src.tensor, src.offset, [src.ap[0], [1, k], [0, P]])
            nc.vector.tensor_tensor(
                out=selB[:, 0:k * P].rearrange("p (s d) -> p s d", d=P),
                in0=iota_b8[:, 0:k * P].rearrange("p (s d) -> p s d", d=P),
                in1=bc,
                op=mybir.AluOpType.is_equal)
            return selB

        def sel_chunks(work):
            """Group (col, slot) pairs into runs of consecutive cols (<=SELBLK)."""
            runs = []
            for (col, slot) in work:
                if runs and col == runs[-1][0] + len(runs[-1][1]) \
                        and len(runs[-1][1]) < SELBLK:
                    runs[-1][1].append(slot)
                else:
                    runs.append((col, [slot]))
            return runs

        def gather_batch(gp, bat, table_of_bank):
            nb = bat["slot_hi"] - bat["slot_lo"]
            gbuf = gp.tile([P, nb * P], bf16, tag="gbuf")
            for (slo, nsl, b) in bat["calls"]:
                ni = nsl * P
                lo = slo - bat["slot_lo"]
                nc.gpsimd.dma_gather(
                    out_ap=gbuf[:, lo * P:(lo + nsl) * P]
                        .rearrange("p (c f) -> p c f", f=P),
                    in_ap=table_of_bank(b),
                    idxs_ap=g16_sb[:, slo * 8:(slo + nsl) * 8],
                    num_idxs=ni, num_idxs_reg=ni, elem_size=P,
                    single_packet=os.environ.get("GCN_SINGLE_PACKET", "1") == "1",
                    queue_num=next_q(),
                )
            return gbuf

        # ---------------- layer 1 ----------------
        GB = int(os.environ.get("GCN_GBUFS", "4"))
        with tc.tile_pool(name="l1gather", bufs=GB) as gp, \
             tc.tile_pool(name="l1sel", bufs=4) as selp, \
             tc.tile_pool(name="l1work", bufs=3) as wp, \
             tc.tile_pool(name="l1agg_ps", bufs=2, space="PSUM") as agg_ps, \
             tc.tile_pool(name="l1o1_ps", bufs=2, space="PSUM") as o1_ps, \
             tc.tile_pool(name="l1t_ps", bufs=2, space="PSUM") as t_ps, \
             tc.tile_pool(name="l1hw_ps", bufs=2, space="PSUM") as hw_ps:
            for bat in pl.batches:
                nt = bat["tile_hi"] - bat["tile_lo"]
                xloc_sb = gp.tile([P, nt * F], bf16, tag="xloc")
                nc.sync.dma_start(
                    out=xloc_sb[:],
                    in_=xsloc_p[:, bat["tile_lo"] * F:bat["tile_hi"] * F])
                gbuf = gather_batch(
                    gp, bat,
                    lambda b: xs_p[b * BK:b * BK + bank_rows[b], :])
                for i in range(bat["tile_lo"], bat["tile_hi"]):
                    psum_agg = agg_ps.tile([P, P], f32, name="psum_agg")
                    work = bat["tile_work"][i]
                    tot = len(work)
                    # self-loop: agg[f, d] += xs[d, f]
                    lofs = (i - bat["tile_lo"]) * F
                    nc.tensor.matmul(
                        out=psum_agg[:],
                        lhsT=xloc_sb[:, lofs:lofs + F],
                        rhs=ident_b[:],
                        start=True, stop=(tot == 0))
                    done = 0
                    for (col0, slots) in sel_chunks(work):
                        selB = sel_build_run(selp, col0, len(slots))
                        for j, slot in enumerate(slots):
                            cofs = (slot - bat["slot_lo"]) * P
                            nc.tensor.matmul(
                                out=psum_agg[:],
                                lhsT=gbuf[:, cofs:cofs + F],
                                rhs=selB[:, j * P:(j + 1) * P],
                                start=False,
                                stop=(done == tot - 1),
                            )
                            done += 1
                    agg_sb = wp.tile([P, P], bf16, name="agg_sb")
                    nc.scalar.activation(
                        agg_sb[:], psum_agg[:],
                        mybir.ActivationFunctionType.Copy)
                    # psum_o1[d, H] = agg^T @ W1 + dcinv (x) b1
                    psum_o1 = o1_ps.tile([P, H], f32, name="psum_o1")
                    nc.tensor.matmul(out=psum_o1[:], lhsT=agg_sb[:],
                                     rhs=w1_sb[:], start=True, stop=False)
                    nc.tensor.matmul(out=psum_o1[:],
                                     lhsT=dcinv_sb[0:1, i * P:(i + 1) * P],
                                     rhs=b1_sb[:], start=False, stop=True)
                    # h[d, H] = relu(dis_c * psum)
                    h_sb = wp.tile([P, H], bf16, name="h_sb")
                    nc.scalar.activation(
                        h_sb[:], psum_o1[:],
                        mybir.ActivationFunctionType.Relu,
                        bias=0.0, scale=disc_sb[:, i:i + 1])
                    # transpose h -> [H, d]
                    psum_t = t_ps.tile([H, P], bf16, name="psum_t")
                    nc.tensor.transpose(psum_t[:], h_sb[:], ident_b[:])
                    ht_sb = wp.tile([H, P], bf16, name="ht_sb")
                    nc.scalar.activation(
                        ht_sb[:], psum_t[:],
                        mybir.ActivationFunctionType.Copy)
                    # hw[d, C] = dis_c * (h @ W2)
                    psum_hw = hw_ps.tile([P, C], f32, name="psum_hw")
                    nc.tensor.matmul(out=psum_hw[:], lhsT=ht_sb[:],
                                     rhs=w2_sb[:], start=True, stop=True)
                    nc.scalar.activation(
                        hwall_sb[:, i * C:(i + 1) * C], psum_hw[:],
                        mybir.ActivationFunctionType.Copy,
                        scale=disc_sb[:, i:i + 1])
                    nc.sync.dma_start(
                        out=hw_ag_in[i * P:(i + 1) * P, :],
                        in_=hwall_sb[:, i * C:(i + 1) * C])

        # ------------- all-gather + repack -------------
        nc.gpsimd.collective_compute(
            "AllGather",
            mybir.AluOpType.bypass,
            replica_groups=[list(range(M))],
            ins=[hw_ag_in[:, :]],
            outs=[hw_ag_out[:, :]],
        )
        # repack bank by bank (half-bank chunks) so gathers start early
        HB = 16384  # rows per half-chunk (= 128 x 128)
        with tc.tile_pool(name="rp_in", bufs=2) as rpi, \
             tc.tile_pool(name="rp_out", bufs=2) as rpo:
            for b in range(pl.NB):
                r0 = 0
                while r0 < bank_rows[b]:
                    n = min(HB, bank_rows[b] - r0)
                    rpp = n // P  # rows per partition
                    tin = rpi.tile([P, HB // P * C], bf16, name="rp_in")
                    src_ap = hw_ag_out[b * BK + r0:b * BK + r0 + n, :] \
                        .rearrange("(p r) f -> p (r f)", p=P)
                    nc.sync.dma_start(out=tin[:, 0:rpp * C], in_=src_ap)
                    tout = rpo.tile([P, HB // P * P], bf16, name="rp_out")
                    nc.vector.tensor_copy(
                        out=tout[:, 0:rpp * P]
                            .rearrange("p (r f) -> p r f", f=P)[:, :, 0:C],
                        in_=tin[:, 0:rpp * C].rearrange("p (r f) -> p r f", f=C))
                    dst_ap = hw_tab_b[b][r0:r0 + n, :].rearrange(
                        "(p r) f -> p (r f)", p=P)
                    nc.sync.dma_start(out=dst_ap, in_=tout[:, 0:rpp * P])
                    r0 += n

        # ---------------- layer 2 ----------------
        with tc.tile_pool(name="l2gather", bufs=GB) as gp2, \
             tc.tile_pool(name="l2sel", bufs=4) as selp2, \
             tc.tile_pool(name="l2work", bufs=3) as wp2, \
             tc.tile_pool(name="l2o2_ps", bufs=4, space="PSUM") as o2_ps:
            for bat in pl.batches:
                gbuf2 = gather_batch(gp2, bat, lambda b: hw_tab_b[b][:, :])
                for i in range(bat["tile_lo"], bat["tile_hi"]):
                    psum_o2 = o2_ps.tile([P, C], f32, name="psum_o2")
                    # self-loop: out2[d, c] += hw[d, c]
                    nc.tensor.matmul(
                        out=psum_o2[:],
                        lhsT=ident_b[:],
                        rhs=hwall_sb[:, i * C:(i + 1) * C],
                        start=True, stop=False)
                    for (col0, slots) in sel_chunks(bat["tile_work"][i]):
                        selB = sel_build_run(selp2, col0, len(slots))
                        for j, slot in enumerate(slots):
                            cofs = (slot - bat["slot_lo"]) * P
                            nc.tensor.matmul(
                                out=psum_o2[:],
                                lhsT=selB[:, j * P:(j + 1) * P],
                                rhs=gbuf2[:, cofs:cofs + C],
                                start=False,
                                stop=False,
                            )
                    nc.tensor.matmul(out=psum_o2[:],
                                     lhsT=dcinv_sb[0:1, i * P:(i + 1) * P],
                                     rhs=b2_sb[:], start=False, stop=True)
                    nc.scalar.activation(
                        outall_sb[:, i * C:(i + 1) * C], psum_o2[:],
                        mybir.ActivationFunctionType.Copy,
                        scale=disc_sb[:, i:i + 1])
            nc.sync.dma_start(out=out_p[:, :], in_=outall_sb[:])

    nc.compile()
    return nc


# ---------------------------------------------------------------------------
# Input packing / output unpacking
# ---------------------------------------------------------------------------
def make_in_maps(pl, x, W1, b1, W2, b2):
    import ml_dtypes
    bf = ml_dtypes.bfloat16
    x = np.asarray(x, dtype=np.float32)
    dis = pl.dis
    N, M, Nc, GR, NR, F = pl.N, pl.M, pl.Nc, pl.GR, pl.NR, pl.F
    xs = np.zeros((NR, F), dtype=bf)
    xsc = (x * dis[:, None]).astype(bf)
    for c in range(M):
        lo = c * Nc
        hi = min(lo + Nc, N)
        xs[c * GR:c * GR + (hi - lo)] = xsc[lo:hi]
    W1b = np.ascontiguousarray(np.asarray(W1, np.float32).astype(bf))
    b1b = np.ascontiguousarray(np.asarray(b1, np.float32).reshape(1, -1).astype(bf))
    W2b = np.ascontiguousarray(np.asarray(W2, np.float32).astype(bf))
    b2b = np.ascontiguousarray(np.asarray(b2, np.float32).reshape(1, -1).astype(bf))
    T = pl.T
    in_maps = []
    for c in range(M):
        xsl = xs[c * GR:(c + 1) * GR].reshape(T, P, F).transpose(1, 0, 2)
        in_maps.append({
            "xs": xs,
            "xsloc": np.ascontiguousarray(xsl.reshape(P, T * F)),
            "W1b": W1b, "b1b": b1b, "W2b": W2b, "b2b": b2b,
            "colsel": np.ascontiguousarray(pl.colsel[c].astype(bf)),
            "g16": np.ascontiguousarray(pl.gidx16[c]),
            "disc": np.ascontiguousarray(pl.dis_c[c]),
            "dcinv": np.ascontiguousarray(pl.dcinv[c].astype(bf)),
        })
    return in_maps


def unpack_outputs(pl, outs):
    T, C = pl.T, pl.C
    parts = []
    for o in outs:
        o = np.asarray(o).reshape(P, T, C).transpose(1, 0, 2).reshape(T * P, C)
        parts.append(o[:pl.Nc])
    return np.concatenate(parts, axis=0)[:pl.N]


# ---------------------------------------------------------------------------
# Public entry point
# ---------------------------------------------------------------------------
_CACHE = {}


def _get_compiled(edge_index, n_nodes, f_in, hidden, n_class, n_cores=8):
    key = (edge_index.shape, n_nodes, f_in, hidden, n_class, n_cores,
           int(np.asarray(edge_index[0, :8]).sum()),
           int(np.asarray(edge_index[1, -8:]).sum()))
    hit = _CACHE.get(key)
    if hit is None:
        pl = make_plan(edge_index, n_nodes, n_cores, f_in, hidden, n_class)
        nc = build_program(pl)
        _CACHE[key] = hit = (pl, nc)
    return hit


def kernel(x, edge_index, W1, b1, W2, b2):
    from concourse import bass_utils

    x = np.asarray(x)
    edge_index = np.asarray(edge_index)
    n_nodes, f_in = x.shape
    hidden = np.asarray(W1).shape[1]
    n_class = np.asarray(W2).shape[1]
    n_cores = 8

    pl, nc = _get_compiled(edge_index, n_nodes, f_in, hidden, n_class, n_cores)
    in_maps = make_in_maps(pl, x, W1, b1, W2, b2)
    res = bass_utils.run_bass_kernel_spmd(
        nc, in_maps, core_ids=list(range(n_cores)))
    kernel.last_exec_time_ns = res.exec_time_ns
    kernel.last_result = res
    outs = [res.results[c]["out"] for c in range(n_cores)]
    return unpack_outputs(pl, outs)
